# revision 26
# baseline (speedup 1.0000x reference)
"""Channel-attention kernel for Trainium2 (8 NeuronCores, SPMD).

Reference computation (B=2, C=512, H=W=64, heads=8, hd=64, N=H*W=4096):
    tokens = x.transpose(0,2,3,1).reshape(B,N,C)
    qkv    = tokens @ w_qkv.T -> q,k,v per head    (k scaled by hd**-0.5)
    attn   = softmax(k @ v.T, axis=-1)             # [B,h,N,N]
    out    = attn @ q                              # [B,h,N,hd]
    out -> (B,N,h,hd) -> (B,H,W,C) -> (B,C,H,W) -> reshape (B,N,C)   (raw
           reinterpretation; mixes channel/spatial)
    y      = out @ w_proj.T + b_proj -> reshape (B,C,H,W)

Key structural facts exploited here:
  * The odd (B,C,H,W)->(B,N,C) reinterpretation makes row jj of the proj
    input equal to A[jj//8, (jj%8)*512 : (jj%8)*512+512] where A is the
    attention output in channel-major [C, N] layout -- the whole network is
    head-separable end to end, including the projection.
  * Likewise the final (B,N,C)->(B,C,H,W) reshape means the per-head
    proj output Y[jj, c''] laid out row-major IS the output channel block
    [h*64:(h+1)*64] of the (C,H,W) tensor.

Sharding: 16 (batch, head) pairs over 8 cores -> each core handles one
batch element and two adjacent heads.  Weights are sliced per head pair
and pre-transposed on the host (cheap); all heavy compute runs on device.

Performance notes (HW-measured):
  * The PE clock is HAM-gated: any >3.4us idle gap on the array halves the
    clock for the next ~7us.  The kernel therefore (a) spins warm-up
    transposes at t=0 while the input DMAs stream, and (b) software-
    pipelines chunk boundaries so the PE never stalls on the normalize
    tail (the normalize of chunk j is emitted in the middle of chunk
    j+1's stream, never at its head).
  * S and O matmuls are both stream-port-bound (1 column/cycle), so the
    matmuls are organized as plain 512-col streams; two heads' S matmuls
    use disjoint PE row groups so LDWEIGHTS hides under the other head's
    stream.
  * exp() of the 33.5M attention scores is the 2nd-largest cost; it is
    split between ACT (true exp) and DVE (Schraudolph bit-trick exp)
    with a build-time running balance that also accounts for the copies
    each engine does.
"""

import os

import ml_dtypes
import numpy as np

import concourse.bass as bass
import concourse.mybir as mybir
import concourse.tile as tile
from concourse import bacc, bass_utils
from concourse.bass import ts
from concourse.masks import make_identity

F32 = mybir.dt.float32
BF16 = mybir.dt.bfloat16
I16 = mybir.dt.int16
EXP = mybir.ActivationFunctionType.Exp
# Schraudolph fast-exp in the bf16 domain: exp(x) ~= bitcast_bf16(int16(
# A*x + B)).  Max rel err ~4.2%, mean tuned to ~0; the sawtooth error
# washes out after the softmax weighted average over 4096 terms.
SCH_A = float(2 ** 7 / np.log(2.0))
SCH_B = float(127 * 2 ** 7 - 479765 / 65536)

B, C, H, W = 2, 512, 64, 64
N = H * W                 # 4096
HEADS_TOTAL = 8
HD = C // HEADS_TOTAL     # 64
SCALE = HD ** -0.5
N_CORES = 8
HPC = 2                   # heads per core
NB = N // 128             # 32 m-blocks
NJ2 = N // 1024           # 4 n-superchunks
CC = C // 128             # 4 contraction chunks

# per-instruction engine costs (ns) for the build-time exp balance
ACT_EXP = (172 + 1536) / 1.2
DVE_EXP = (120 + 1536) / 0.96
ACT_OCP = (172 + 512) / 1.2
DVE_OCP = (120 + 512) / 0.96
DVE_TAIL = (120 + 4) / 0.96 + 4 * (120 + 64) / 0.96


def _emit(nc, tc):
    x_h = nc.dram_tensor("x", [C, N], BF16, kind="ExternalInput")
    wq_h = nc.dram_tensor("wq", [C, 128], BF16, kind="ExternalInput")
    wk_h = nc.dram_tensor("wk", [C, 128], BF16, kind="ExternalInput")
    wv_h = nc.dram_tensor("wv", [C, 128], BF16, kind="ExternalInput")
    wp_h = nc.dram_tensor("wp", [C, C], BF16, kind="ExternalInput")
    bp_h = nc.dram_tensor("bp", [1, C], F32, kind="ExternalInput")
    out_h = nc.dram_tensor("out", [HPC, 512, 512], F32, kind="ExternalOutput")

    singles = tc.alloc_tile_pool(name="singles", bufs=1)
    epool = tc.alloc_tile_pool(name="epool", bufs=6)
    ojpool = tc.alloc_tile_pool(name="ojpool", bufs=2)
    vpool = tc.alloc_tile_pool(name="vpool", bufs=4)
    spool = tc.alloc_tile_pool(name="spool", bufs=2, space="PSUM")   # 6 banks
    opool = tc.alloc_tile_pool(name="opool", bufs=2, space="PSUM")   # 2 banks

    # ---- persistent SBUF tensors ----
    x_sb = singles.tile([128, CC, N], BF16)        # x[cc*128+p, n]
    wq_sb = singles.tile([128, CC, 128], BF16)
    wk_sb = singles.tile([128, CC, 128], BF16)
    wv_sb = singles.tile([128, CC, 128], BF16)
    wp_sb = singles.tile([128, CC, 512], BF16)
    bias_sb = singles.tile([128, 512], F32)
    id_sb = singles.tile([128, 128], BF16)
    kT_sb = singles.tile([128, N], BF16)           # [2*64 ch, n]
    vT_sb = singles.tile([128, N], BF16)
    # q token-major, both heads side by side, each with its own ones col:
    # cols [65h : 65h+64] = q of head h, col 65h+64 = 1.0
    qa_sb = singles.tile([128, NB, 130], BF16)
    # proj lhsT: mt[h][p, kk, jj]: contraction c = kk*128+p (n-offset inside
    # the jj%8-th 512-chunk of the channel), free jj = 8*ch + (n-chunk)
    mt_sb = [singles.tile([128, CC, 512], BF16, name=f"mt{h}") for h in range(HPC)]

    make_identity(nc, id_sb)
    nc.vector.memset(qa_sb[:, :, 64:65], 1.0)
    nc.vector.memset(qa_sb[:, :, 129:130], 1.0)

    # ---- input DMAs ----
    x_view = x_h.ap().rearrange("(cc p) n -> p cc n", p=128)

    def x_quarter(q8):
        # single descriptor per quarter: the sync engine issues DMAs
        # serially, so fewer/larger transfers keep the front DMA-fed
        nc.sync.dma_start(
            out=x_sb[:, :, ts(q8, N // 8)], in_=x_view[:, :, ts(q8, N // 8)]
        )

    nc.sync.dma_start(out=wk_sb, in_=wk_h.ap().rearrange("(cc p) m -> p cc m", p=128))
    x_quarter(0)
    nc.sync.dma_start(out=wv_sb, in_=wv_h.ap().rearrange("(cc p) m -> p cc m", p=128))
    x_quarter(1)
    nc.sync.dma_start(out=wq_sb, in_=wq_h.ap().rearrange("(cc p) m -> p cc m", p=128))
    for q8 in range(2, 8):
        x_quarter(q8)
    nc.sync.dma_start(out=wp_sb, in_=wp_h.ap().rearrange("(cc p) m -> p cc m", p=128))
    nc.sync.dma_start(out=bias_sb, in_=bp_h.ap().to_broadcast((128, 512)))

    # ---- PE warm-up ----
    # The HAM clock gate starts at half clock and needs ~3.4us of sustained
    # array activity to release.  Spin transposes of the identity (no data
    # deps beyond make_identity) while the x DMAs stream, so the KV phase
    # starts at full clock.
    if os.environ.get("K_NO_WARMUP") != "1":
        for _ in range(30):
            wu_ps = spool.tile([128, 128], BF16, tag="s", name="wu_ps")
            nc.tensor.transpose(wu_ps, id_sb, id_sb)

    # ---- QKV phase ----
    # Packed per x-quarter so the PE always has work while the next
    # quarter's DMA is still in flight: K^T chunk, V^T chunk, then the
    # four Q m-blocks of the same quarter.
    for j8 in range(8):
        for w_sb, dst in ((wk_sb, kT_sb), (wv_sb, vT_sb)):
            kv_ps = spool.tile([128, 512], F32, tag="s", name="kv_ps")
            for cc in range(CC):
                nc.tensor.matmul(
                    kv_ps,
                    lhsT=w_sb[:, cc, :],
                    rhs=x_sb[:, cc, ts(j8, 512)],
                    start=(cc == 0),
                    stop=(cc == CC - 1),
                )
            nc.scalar.copy(out=dst[:, ts(j8, 512)], in_=kv_ps)
        for nb in range(4 * j8, 4 * j8 + 4):
            q_ps = spool.tile([128, 128], F32, tag="s", name="q_ps")
            for cc in range(CC):
                nc.tensor.matmul(
                    q_ps,
                    lhsT=x_sb[:, cc, ts(nb, 128)],
                    rhs=wq_sb[:, cc, :],
                    start=(cc == 0),
                    stop=(cc == CC - 1),
                )
            # [128, 2, 64] strided copy: head h -> qa cols [65h, 65h+64)
            nc.vector.tensor_copy(
                out=qa_sb[:, nb, :].rearrange("p (g c) -> p g c", g=2)[:, :, 0:64],
                in_=q_ps.rearrange("p (g c) -> p g c", g=2),
            )

    # ---- attention + norm ----
    # Baseline-proven stream structure: groups of GRP=3 (head, m-block)
    # tiles with heads alternating per tile (each S LDWEIGHTS targets the
    # other PE row group and hides under the current stream), one exp per
    # group, O matmuls lagged two groups.  On top of that: the O
    # accumulators are drained immediately at each chunk boundary and the
    # normalize tail (transposes + 1/Z) runs right there, so the next
    # chunk's stream starts without a long PE stall and the HAM clock
    # never re-throttles mid-attention.
    GRP = 3
    NT = 2 * NB
    n_grp = (NT + GRP - 1) // GRP
    eng_t = [0.0, 0.0]   # virtual ns on (ACT, DVE)

    def exp_to(e, s_ps, force=None):
        if force == 0 or (force is None and eng_t[0] <= eng_t[1]):
            eng_t[0] += ACT_EXP
            nc.scalar.activation(out=e, in_=s_ps, func=EXP)
        else:
            eng_t[1] += DVE_EXP
            nc.vector.tensor_scalar(
                out=e.bitcast(I16),
                in0=s_ps,
                scalar1=SCH_A,
                scalar2=SCH_B,
                op0=mybir.AluOpType.mult,
                op1=mybir.AluOpType.add,
            )

    def emit_tail(h, j, oj):
        # transposes of the drained O chunk + 1/Z application into mt
        t_ps = opool.tile([128, 4, 66], BF16, tag="o", name="t_ps")
        for q32 in range(4):
            nc.tensor.transpose(
                t_ps[:, q32, 0:65], oj[:, ts(q32, 128)], id_sb[0:65, 0:65]
            )
        rz4 = vpool.tile([128, 4], F32, tag="rz", name="rz4")
        nc.vector.reciprocal(out=rz4, in_=t_ps[:, :, 64:65])
        eng_t[1] += DVE_TAIL
        for q32 in range(4):
            dst = mt_sb[h][:, q32, :].rearrange("p (ch p8) -> p ch p8", p8=8)[:, :, j]
            nc.vector.tensor_scalar_mul(
                dst, t_ps[:, q32, 0:64], rz4[:, q32:q32 + 1]
            )

    NJ = 8
    pend_tails = []
    for j in range(NJ):
        o_ps = None
        e_tiles = []

        def emit_o(g):
            g0, glen, pe = e_tiles[g]
            for t in range(glen):
                k = g0 + t
                h, i = k % 2, k // 2
                nc.tensor.matmul(
                    o_ps[h][0:65, :],
                    lhsT=qa_sb[:, i, 65 * h:65 * h + 65],
                    rhs=pe[:, t, :],
                    start=(i == 0),
                    stop=(i == NB - 1),
                )

        for g in range(n_grp):
            g0 = g * GRP
            glen = min(GRP, NT - g0)
            s_ps = spool.tile([128, GRP, 512], F32, tag="s", name="s_ps")
            for t in range(glen):
                k = g0 + t
                h, i = k % 2, k // 2
                hb = h * HD
                nc.tensor.matmul(
                    s_ps[:, t, :],
                    lhsT=vT_sb[hb:hb + HD, ts(i, 128)],
                    rhs=kT_sb[hb:hb + HD, ts(j, 512)],
                    start=True,
                    stop=True,
                )
            e = epool.tile([128, GRP, 512], BF16, tag="e", name="e")
            # leading groups on ACT (so the boundary oj copies clear fast);
            # the last two groups split one-per-engine so the 2-deep spool
            # rotation never gates the next chunk's S stream on one queue.
            if g < 2:
                force = 0
            elif g == n_grp - 2:
                force = 1
            elif g == n_grp - 1:
                force = 0
            else:
                force = None
            exp_to(e[:, 0:glen, :], s_ps[:, 0:glen, :], force=force)
            e_tiles.append((g0, glen, e))
            if g == 1 and pend_tails:
                # normalize tail of the previous chunk, two S-groups into
                # this chunk's stream: the oj copies have completed by now
                # and the PE queue behind has plenty of work.
                for t_ in pend_tails:
                    emit_tail(*t_)
                pend_tails = []
            if g == 2:
                # allocate AFTER the tails' t_ps tiles so the pool rotation
                # order matches the execution order
                o_ps = [opool.tile([128, 512], F32, tag="o", name=f"o_{h}")
                        for h in range(HPC)]
            if g >= 4 and g % 2 == 0:
                emit_o(g - 4)
                emit_o(g - 3)
        for gg in range(n_grp - 4, n_grp):
            emit_o(gg)
        # drain the O accumulators immediately (on ACT mid-stream: it has
        # no boundary backlog thanks to the forced-ACT leading groups; on
        # DVE for the last chunk, whose trailing exps were forced to ACT,
        # so the final tails start without queueing behind an exp)
        for h in range(HPC):
            oj = ojpool.tile([65, 512], BF16, tag="oj", name=f"oj{h}")
            if j == NJ - 1 or h == 1:
                eng_t[1] += DVE_OCP
                nc.vector.tensor_copy(out=oj, in_=o_ps[h][0:65])
            else:
                eng_t[0] += ACT_OCP
                nc.scalar.copy(out=oj, in_=o_ps[h][0:65])
            pend_tails.append((h, j, oj))

    # ---- projection (interleaved with the last normalize tail) ----
    # y[jj, c''] = sum_c M[jj, c] w_proj[c'', c] + bp[c'']
    def emit_proj(h):
        for l in range(4):
            y_ps = spool.tile([128, 512], F32, tag="s", name="y_ps")
            for kk in range(CC):
                nc.tensor.matmul(
                    y_ps,
                    lhsT=mt_sb[h][:, kk, ts(l, 128)],
                    rhs=wp_sb[:, kk, :],
                    start=(kk == 0),
                    stop=(kk == CC - 1),
                )
            y_sb = vpool.tile([128, 512], F32, tag="y", name="y_sb")
            nc.vector.tensor_add(out=y_sb, in0=y_ps, in1=bias_sb)
            nc.sync.dma_start(out=out_h.ap()[h, ts(l, 128), :], in_=y_sb)

    emit_tail(*pend_tails.pop(0))
    emit_tail(*pend_tails.pop(0))
    emit_proj(0)
    emit_proj(1)

    for pool in (opool, spool, vpool, ojpool, epool, singles):
        pool.release()


_CACHE = {}


def _build():
    if "nc" not in _CACHE:
        nc = bacc.Bacc("TRN2", target_bir_lowering=False, debug=False)
        with tile.TileContext(nc) as tc:
            _emit(nc, tc)
        nc.compile()
        _CACHE["nc"] = nc
    return _CACHE["nc"]


def _shard(x, w_qkv, w_proj, b_proj):
    """Build the 8 per-core input maps from the full inputs."""
    bf16 = ml_dtypes.bfloat16
    wpT = np.ascontiguousarray(w_proj.T).astype(bf16)
    bp = np.ascontiguousarray(b_proj.reshape(1, C))
    in_maps = []
    for core in range(N_CORES):
        b = core // 4
        h0 = HPC * (core % 4)
        r0 = h0 * HD
        in_maps.append({
            "x": np.ascontiguousarray(x[b].reshape(C, N)).astype(bf16),
            "wq": np.ascontiguousarray(w_qkv[r0:r0 + 128, :].T).astype(bf16),
            "wk": np.ascontiguousarray(
                (w_qkv[C + r0:C + r0 + 128, :] * SCALE).T).astype(bf16),
            "wv": np.ascontiguousarray(
                w_qkv[2 * C + r0:2 * C + r0 + 128, :].T).astype(bf16),
            "wp": wpT,
            "bp": bp,
        })
    return in_maps


def _gather(results):
    full = np.empty((B, C, N), dtype=np.float32)
    for core in range(N_CORES):
        b = core // 4
        h0 = HPC * (core % 4)
        y = results[core]["out"]  # [2, 512, 512]
        for hi in range(HPC):
            ch0 = (h0 + hi) * HD
            full[b, ch0:ch0 + HD] = y[hi].reshape(HD, N)
    return full.reshape(B, C, H, W)


def run(inputs, trace=False, **kw):
    nc = _build()
    in_maps = _shard(**inputs)
    res = bass_utils.run_bass_kernel_spmd(
        nc, in_maps, core_ids=list(range(N_CORES)), trace=trace, **kw
    )
    return _gather(res.results), res


def kernel(x, w_qkv, w_proj, b_proj):
    out, _ = run(dict(x=x, w_qkv=w_qkv, w_proj=w_proj, b_proj=b_proj))
    return out


# revision 28
# speedup vs baseline: 1.0125x; 1.0125x over previous
"""Channel-attention kernel for Trainium2 (8 NeuronCores, SPMD).

Reference computation (B=2, C=512, H=W=64, heads=8, hd=64, N=H*W=4096):
    tokens = x.transpose(0,2,3,1).reshape(B,N,C)
    qkv    = tokens @ w_qkv.T -> q,k,v per head    (k scaled by hd**-0.5)
    attn   = softmax(k @ v.T, axis=-1)             # [B,h,N,N]
    out    = attn @ q                              # [B,h,N,hd]
    out -> (B,N,h,hd) -> (B,H,W,C) -> (B,C,H,W) -> reshape (B,N,C)   (raw
           reinterpretation; mixes channel/spatial)
    y      = out @ w_proj.T + b_proj -> reshape (B,C,H,W)

Key structural facts exploited here:
  * The odd (B,C,H,W)->(B,N,C) reinterpretation makes row jj of the proj
    input equal to A[jj//8, (jj%8)*512 : (jj%8)*512+512] where A is the
    attention output in channel-major [C, N] layout -- the whole network is
    head-separable end to end, including the projection.
  * Likewise the final (B,N,C)->(B,C,H,W) reshape means the per-head
    proj output Y[jj, c''] laid out row-major IS the output channel block
    [h*64:(h+1)*64] of the (C,H,W) tensor.

Sharding: 16 (batch, head) pairs over 8 cores -> each core handles one
batch element and two adjacent heads.  Weights are sliced per head pair
and pre-transposed on the host (cheap); all heavy compute runs on device.

Performance notes (HW-measured):
  * The PE clock is HAM-gated: any >3.4us idle gap on the array halves the
    clock for the next ~7us.  The kernel therefore (a) spins warm-up
    transposes at t=0 while the input DMAs stream, and (b) software-
    pipelines chunk boundaries so the PE never stalls on the normalize
    tail (the normalize of chunk j is emitted in the middle of chunk
    j+1's stream, never at its head).
  * S and O matmuls are both stream-port-bound (1 column/cycle), so the
    matmuls are organized as plain 512-col streams; two heads' S matmuls
    use disjoint PE row groups so LDWEIGHTS hides under the other head's
    stream.
  * exp() of the 33.5M attention scores is the 2nd-largest cost; it is
    split between ACT (true exp) and DVE (Schraudolph bit-trick exp)
    with a build-time running balance that also accounts for the copies
    each engine does.
"""

import os

import ml_dtypes
import numpy as np

import concourse.bass as bass
import concourse.mybir as mybir
import concourse.tile as tile
from concourse import bacc, bass_utils
from concourse.bass import ts
from concourse.masks import make_identity

F32 = mybir.dt.float32
BF16 = mybir.dt.bfloat16
I16 = mybir.dt.int16
EXP = mybir.ActivationFunctionType.Exp
# Schraudolph fast-exp in the bf16 domain: exp(x) ~= bitcast_bf16(int16(
# A*x + B)).  Max rel err ~4.2%, mean tuned to ~0; the sawtooth error
# washes out after the softmax weighted average over 4096 terms.
SCH_A = float(2 ** 7 / np.log(2.0))
SCH_B = float(127 * 2 ** 7 - 479765 / 65536)

B, C, H, W = 2, 512, 64, 64
N = H * W                 # 4096
HEADS_TOTAL = 8
HD = C // HEADS_TOTAL     # 64
SCALE = HD ** -0.5
N_CORES = 8
HPC = 2                   # heads per core
NB = N // 128             # 32 m-blocks
NJ2 = N // 1024           # 4 n-superchunks
CC = C // 128             # 4 contraction chunks

# per-instruction engine costs (ns) for the build-time exp balance
ACT_EXP = (172 + 1536) / 1.2
DVE_EXP = (120 + 1536) / 0.96
ACT_OCP = (172 + 512) / 1.2
DVE_OCP = (120 + 512) / 0.96
DVE_TAIL = (120 + 4) / 0.96 + 4 * (120 + 64) / 0.96


def _emit(nc, tc):
    x_h = nc.dram_tensor("x", [C, N], BF16, kind="ExternalInput")
    wq_h = nc.dram_tensor("wq", [C, 128], BF16, kind="ExternalInput")
    wk_h = nc.dram_tensor("wk", [C, 128], BF16, kind="ExternalInput")
    wv_h = nc.dram_tensor("wv", [C, 128], BF16, kind="ExternalInput")
    wp_h = nc.dram_tensor("wp", [C, C], BF16, kind="ExternalInput")
    bp_h = nc.dram_tensor("bp", [1, C], F32, kind="ExternalInput")
    out_h = nc.dram_tensor("out", [HPC, 512, 512], F32, kind="ExternalOutput")

    singles = tc.alloc_tile_pool(name="singles", bufs=1)
    epool = tc.alloc_tile_pool(name="epool", bufs=8)
    ojpool = tc.alloc_tile_pool(name="ojpool", bufs=2)
    vpool = tc.alloc_tile_pool(name="vpool", bufs=4)
    spool = tc.alloc_tile_pool(name="spool", bufs=2, space="PSUM")   # 6 banks
    opool = tc.alloc_tile_pool(name="opool", bufs=2, space="PSUM")   # 2 banks

    # ---- persistent SBUF tensors ----
    x_sb = singles.tile([128, CC, N], BF16)        # x[cc*128+p, n]
    wq_sb = singles.tile([128, CC, 128], BF16)
    wk_sb = singles.tile([128, CC, 128], BF16)
    wv_sb = singles.tile([128, CC, 128], BF16)
    wp_sb = singles.tile([128, CC, 512], BF16)
    bias_sb = singles.tile([128, 512], F32)
    id_sb = singles.tile([128, 128], BF16)
    kT_sb = singles.tile([128, N], BF16)           # [2*64 ch, n]
    vT_sb = singles.tile([128, N], BF16)
    # q token-major, both heads side by side, each with its own ones col:
    # cols [65h : 65h+64] = q of head h, col 65h+64 = 1.0
    qa_sb = singles.tile([128, NB, 130], BF16)
    # proj lhsT: mt[h][p, kk, jj]: contraction c = kk*128+p (n-offset inside
    # the jj%8-th 512-chunk of the channel), free jj = 8*ch + (n-chunk)
    mt_sb = [singles.tile([128, CC, 512], BF16, name=f"mt{h}") for h in range(HPC)]

    make_identity(nc, id_sb)
    nc.vector.memset(qa_sb[:, :, 64:65], 1.0)
    nc.vector.memset(qa_sb[:, :, 129:130], 1.0)

    # ---- input DMAs ----
    x_view = x_h.ap().rearrange("(cc p) n -> p cc n", p=128)

    def x_quarter(q8):
        # single descriptor per quarter: the sync engine issues DMAs
        # serially, so fewer/larger transfers keep the front DMA-fed
        nc.sync.dma_start(
            out=x_sb[:, :, ts(q8, N // 8)], in_=x_view[:, :, ts(q8, N // 8)]
        )

    nc.sync.dma_start(out=wk_sb, in_=wk_h.ap().rearrange("(cc p) m -> p cc m", p=128))
    x_quarter(0)
    nc.sync.dma_start(out=wv_sb, in_=wv_h.ap().rearrange("(cc p) m -> p cc m", p=128))
    x_quarter(1)
    nc.sync.dma_start(out=wq_sb, in_=wq_h.ap().rearrange("(cc p) m -> p cc m", p=128))
    for q8 in range(2, 8):
        x_quarter(q8)
    nc.sync.dma_start(out=wp_sb, in_=wp_h.ap().rearrange("(cc p) m -> p cc m", p=128))
    nc.sync.dma_start(out=bias_sb, in_=bp_h.ap().to_broadcast((128, 512)))

    # ---- PE warm-up ----
    # The HAM clock gate starts at half clock and needs ~3.4us of sustained
    # array activity to release.  Spin transposes of the identity (no data
    # deps beyond make_identity) while the x DMAs stream, so the KV phase
    # starts at full clock.
    if os.environ.get("K_NO_WARMUP") != "1":
        for _ in range(30):
            wu_ps = spool.tile([128, 128], BF16, tag="s", name="wu_ps")
            nc.tensor.transpose(wu_ps, id_sb, id_sb)

    # ---- QKV phase ----
    # Packed per x-quarter so the PE always has work while the next
    # quarter's DMA is still in flight: K^T chunk, V^T chunk, then the
    # four Q m-blocks of the same quarter.
    for j8 in range(8):
        for w_sb, dst in ((wk_sb, kT_sb), (wv_sb, vT_sb)):
            kv_ps = spool.tile([128, 512], F32, tag="s", name="kv_ps")
            for cc in range(CC):
                nc.tensor.matmul(
                    kv_ps,
                    lhsT=w_sb[:, cc, :],
                    rhs=x_sb[:, cc, ts(j8, 512)],
                    start=(cc == 0),
                    stop=(cc == CC - 1),
                )
            nc.scalar.copy(out=dst[:, ts(j8, 512)], in_=kv_ps)
        for nb in range(4 * j8, 4 * j8 + 4):
            q_ps = spool.tile([128, 128], F32, tag="s", name="q_ps")
            for cc in range(CC):
                nc.tensor.matmul(
                    q_ps,
                    lhsT=x_sb[:, cc, ts(nb, 128)],
                    rhs=wq_sb[:, cc, :],
                    start=(cc == 0),
                    stop=(cc == CC - 1),
                )
            # [128, 2, 64] strided copy: head h -> qa cols [65h, 65h+64)
            nc.vector.tensor_copy(
                out=qa_sb[:, nb, :].rearrange("p (g c) -> p g c", g=2)[:, :, 0:64],
                in_=q_ps.rearrange("p (g c) -> p g c", g=2),
            )

    # ---- attention + norm ----
    # Baseline-proven stream structure: groups of GRP=3 (head, m-block)
    # tiles with heads alternating per tile (each S LDWEIGHTS targets the
    # other PE row group and hides under the current stream), one exp per
    # group, O matmuls lagged two groups.  On top of that: the O
    # accumulators are drained immediately at each chunk boundary and the
    # normalize tail (transposes + 1/Z) runs right there, so the next
    # chunk's stream starts without a long PE stall and the HAM clock
    # never re-throttles mid-attention.
    GRP = 3
    NT = 2 * NB
    n_grp = (NT + GRP - 1) // GRP
    eng_t = [0.0, 0.0]   # virtual ns on (ACT, DVE)

    def exp_to(e, s_ps, force=None):
        if force == 0 or (force is None and eng_t[0] <= eng_t[1]):
            eng_t[0] += ACT_EXP
            nc.scalar.activation(out=e, in_=s_ps, func=EXP)
        else:
            eng_t[1] += DVE_EXP
            nc.vector.tensor_scalar(
                out=e.bitcast(I16),
                in0=s_ps,
                scalar1=SCH_A,
                scalar2=SCH_B,
                op0=mybir.AluOpType.mult,
                op1=mybir.AluOpType.add,
            )

    def emit_tail(h, j, oj):
        # transposes of the drained O chunk + 1/Z application into mt
        t_ps = opool.tile([128, 4, 66], BF16, tag="o", name="t_ps")
        for q32 in range(4):
            nc.tensor.transpose(
                t_ps[:, q32, 0:65], oj[:, ts(q32, 128)], id_sb[0:65, 0:65]
            )
        rz4 = vpool.tile([128, 4], F32, tag="rz", name="rz4")
        nc.vector.reciprocal(out=rz4, in_=t_ps[:, :, 64:65])
        eng_t[1] += DVE_TAIL
        for q32 in range(4):
            dst = mt_sb[h][:, q32, :].rearrange("p (ch p8) -> p ch p8", p8=8)[:, :, j]
            nc.vector.tensor_scalar_mul(
                dst, t_ps[:, q32, 0:64], rz4[:, q32:q32 + 1]
            )

    NJ = 8
    pend_tails = []
    carry = None   # (o_ps, e_tiles, emit_o) of the previous chunk's last 4 groups
    for j in range(NJ):
        e_tiles = []
        o_ps = None

        def emit_o(g, _o, _e):
            g0, glen, pe = _e[g]
            for t in range(glen):
                k = g0 + t
                h, i = k % 2, k // 2
                nc.tensor.matmul(
                    _o[h][0:65, :],
                    lhsT=qa_sb[:, i, 65 * h:65 * h + 65],
                    rhs=pe[:, t, :],
                    start=(i == 0),
                    stop=(i == NB - 1),
                )

        for g in range(n_grp):
            g0 = g * GRP
            glen = min(GRP, NT - g0)
            s_ps = spool.tile([128, GRP, 512], F32, tag="s", name="s_ps")
            for t in range(glen):
                k = g0 + t
                h, i = k % 2, k // 2
                hb = h * HD
                nc.tensor.matmul(
                    s_ps[:, t, :],
                    lhsT=vT_sb[hb:hb + HD, ts(i, 128)],
                    rhs=kT_sb[hb:hb + HD, ts(j, 512)],
                    start=True,
                    stop=True,
                )
            e = epool.tile([128, GRP, 512], BF16, tag="e", name="e")
            if g == n_grp - 2:
                force = 1
            elif g == n_grp - 1:
                force = 0
            else:
                force = None
            exp_to(e[:, 0:glen, :], s_ps[:, 0:glen, :], force=force)
            e_tiles.append((g0, glen, e))
            # software pipeline across the chunk seam: the previous chunk's
            # last 4 O-groups and its PSUM drains slot in between this
            # chunk's first S bursts, so the in-order PE queue never idles
            # at a boundary.
            if g == 0 and carry is not None:
                po, pe_, pemit = carry
                pemit(n_grp - 4, po, pe_)
                pemit(n_grp - 3, po, pe_)
            elif g == 1 and carry is not None:
                po, pe_, pemit = carry
                pemit(n_grp - 2, po, pe_)
                pemit(n_grp - 1, po, pe_)
                for h in range(HPC):
                    oj = ojpool.tile([65, 512], BF16, tag="oj", name=f"oj{h}")
                    if h == 0:
                        eng_t[0] += ACT_OCP
                        nc.scalar.copy(out=oj, in_=po[h][0:65])
                    else:
                        eng_t[1] += DVE_OCP
                        nc.vector.tensor_copy(out=oj, in_=po[h][0:65])
                    pend_tails.append((h, j - 1, oj))
                carry = None
            elif g == 2 and pend_tails:
                for t_ in pend_tails:
                    emit_tail(*t_)
                pend_tails = []
            if g == 3:
                # after the tails' t_ps allocations, matching execution order
                o_ps = [opool.tile([128, 512], F32, tag="o", name=f"o_{h}")
                        for h in range(HPC)]
            if g >= 4 and g % 2 == 0:
                emit_o(g - 4, o_ps, e_tiles)
                emit_o(g - 3, o_ps, e_tiles)
        carry = (o_ps, e_tiles, emit_o)

    # flush the last chunk's carried work
    po, pe_, pemit = carry
    for gg in range(n_grp - 4, n_grp):
        pemit(gg, po, pe_)
    for h in range(HPC):
        oj = ojpool.tile([65, 512], BF16, tag="oj", name=f"oj{h}")
        eng_t[1 - h] += DVE_OCP if h == 0 else ACT_OCP
        if h == 0:
            nc.vector.tensor_copy(out=oj, in_=po[h][0:65])
        else:
            nc.scalar.copy(out=oj, in_=po[h][0:65])
        pend_tails.append((h, NJ - 1, oj))

    # ---- projection (interleaved with the last normalize tail) ----
    # y[jj, c''] = sum_c M[jj, c] w_proj[c'', c] + bp[c'']
    def emit_proj(h):
        for l in range(4):
            y_ps = spool.tile([128, 512], F32, tag="s", name="y_ps")
            for kk in range(CC):
                nc.tensor.matmul(
                    y_ps,
                    lhsT=mt_sb[h][:, kk, ts(l, 128)],
                    rhs=wp_sb[:, kk, :],
                    start=(kk == 0),
                    stop=(kk == CC - 1),
                )
            y_sb = vpool.tile([128, 512], F32, tag="y", name="y_sb")
            nc.vector.tensor_add(out=y_sb, in0=y_ps, in1=bias_sb)
            nc.sync.dma_start(out=out_h.ap()[h, ts(l, 128), :], in_=y_sb)

    emit_tail(*pend_tails.pop(0))
    emit_tail(*pend_tails.pop(0))
    emit_proj(0)
    emit_proj(1)

    for pool in (opool, spool, vpool, ojpool, epool, singles):
        pool.release()


_CACHE = {}


def _build():
    if "nc" not in _CACHE:
        nc = bacc.Bacc("TRN2", target_bir_lowering=False, debug=False)
        with tile.TileContext(nc) as tc:
            _emit(nc, tc)
        nc.compile()
        _CACHE["nc"] = nc
    return _CACHE["nc"]


def _shard(x, w_qkv, w_proj, b_proj):
    """Build the 8 per-core input maps from the full inputs."""
    bf16 = ml_dtypes.bfloat16
    wpT = np.ascontiguousarray(w_proj.T).astype(bf16)
    bp = np.ascontiguousarray(b_proj.reshape(1, C))
    in_maps = []
    for core in range(N_CORES):
        b = core // 4
        h0 = HPC * (core % 4)
        r0 = h0 * HD
        in_maps.append({
            "x": np.ascontiguousarray(x[b].reshape(C, N)).astype(bf16),
            "wq": np.ascontiguousarray(w_qkv[r0:r0 + 128, :].T).astype(bf16),
            "wk": np.ascontiguousarray(
                (w_qkv[C + r0:C + r0 + 128, :] * SCALE).T).astype(bf16),
            "wv": np.ascontiguousarray(
                w_qkv[2 * C + r0:2 * C + r0 + 128, :].T).astype(bf16),
            "wp": wpT,
            "bp": bp,
        })
    return in_maps


def _gather(results):
    full = np.empty((B, C, N), dtype=np.float32)
    for core in range(N_CORES):
        b = core // 4
        h0 = HPC * (core % 4)
        y = results[core]["out"]  # [2, 512, 512]
        for hi in range(HPC):
            ch0 = (h0 + hi) * HD
            full[b, ch0:ch0 + HD] = y[hi].reshape(HD, N)
    return full.reshape(B, C, H, W)


def run(inputs, trace=False, **kw):
    nc = _build()
    in_maps = _shard(**inputs)
    res = bass_utils.run_bass_kernel_spmd(
        nc, in_maps, core_ids=list(range(N_CORES)), trace=trace, **kw
    )
    return _gather(res.results), res


def kernel(x, w_qkv, w_proj, b_proj):
    out, _ = run(dict(x=x, w_qkv=w_qkv, w_proj=w_proj, b_proj=b_proj))
    return out


# revision 30
# speedup vs baseline: 1.0155x; 1.0030x over previous
"""Channel-attention kernel for Trainium2 (8 NeuronCores, SPMD).

Reference computation (B=2, C=512, H=W=64, heads=8, hd=64, N=H*W=4096):
    tokens = x.transpose(0,2,3,1).reshape(B,N,C)
    qkv    = tokens @ w_qkv.T -> q,k,v per head    (k scaled by hd**-0.5)
    attn   = softmax(k @ v.T, axis=-1)             # [B,h,N,N]
    out    = attn @ q                              # [B,h,N,hd]
    out -> (B,N,h,hd) -> (B,H,W,C) -> (B,C,H,W) -> reshape (B,N,C)   (raw
           reinterpretation; mixes channel/spatial)
    y      = out @ w_proj.T + b_proj -> reshape (B,C,H,W)

Key structural facts exploited here:
  * The odd (B,C,H,W)->(B,N,C) reinterpretation makes row jj of the proj
    input equal to A[jj//8, (jj%8)*512 : (jj%8)*512+512] where A is the
    attention output in channel-major [C, N] layout -- the whole network is
    head-separable end to end, including the projection.
  * Likewise the final (B,N,C)->(B,C,H,W) reshape means the per-head
    proj output Y[jj, c''] laid out row-major IS the output channel block
    [h*64:(h+1)*64] of the (C,H,W) tensor.

Sharding: 16 (batch, head) pairs over 8 cores -> each core handles one
batch element and two adjacent heads.  Weights are sliced per head pair
and pre-transposed on the host (cheap); all heavy compute runs on device.

Performance notes (HW-measured):
  * The PE clock is HAM-gated: any >3.4us idle gap on the array halves the
    clock for the next ~7us.  The kernel therefore (a) spins warm-up
    transposes at t=0 while the input DMAs stream, and (b) software-
    pipelines chunk boundaries so the PE never stalls on the normalize
    tail (the normalize of chunk j is emitted in the middle of chunk
    j+1's stream, never at its head).
  * S and O matmuls are both stream-port-bound (1 column/cycle), so the
    matmuls are organized as plain 512-col streams; two heads' S matmuls
    use disjoint PE row groups so LDWEIGHTS hides under the other head's
    stream.
  * exp() of the 33.5M attention scores is the 2nd-largest cost; it is
    split between ACT (true exp) and DVE (Schraudolph bit-trick exp)
    with a build-time running balance that also accounts for the copies
    each engine does.
"""

import os

import ml_dtypes
import numpy as np

import concourse.bass as bass
import concourse.mybir as mybir
import concourse.tile as tile
from concourse import bacc, bass_utils
from concourse.bass import ts
from concourse.masks import make_identity

F32 = mybir.dt.float32
BF16 = mybir.dt.bfloat16
I16 = mybir.dt.int16
EXP = mybir.ActivationFunctionType.Exp
# Schraudolph fast-exp in the bf16 domain: exp(x) ~= bitcast_bf16(int16(
# A*x + B)).  Max rel err ~4.2%, mean tuned to ~0; the sawtooth error
# washes out after the softmax weighted average over 4096 terms.
SCH_A = float(2 ** 7 / np.log(2.0))
SCH_B = float(127 * 2 ** 7 - 479765 / 65536)

B, C, H, W = 2, 512, 64, 64
N = H * W                 # 4096
HEADS_TOTAL = 8
HD = C // HEADS_TOTAL     # 64
SCALE = HD ** -0.5
N_CORES = 8
HPC = 2                   # heads per core
NB = N // 128             # 32 m-blocks
NJ2 = N // 1024           # 4 n-superchunks
CC = C // 128             # 4 contraction chunks

# per-instruction engine costs (ns) for the build-time exp balance
ACT_EXP = (172 + 1536) / 1.2
DVE_EXP = (120 + 1536) / 0.96
ACT_OCP = (172 + 512) / 1.2
DVE_OCP = (120 + 512) / 0.96
DVE_TAIL = (120 + 4) / 0.96 + 4 * (120 + 64) / 0.96


def _emit(nc, tc):
    x_h = nc.dram_tensor("x", [C, N], BF16, kind="ExternalInput")
    wq_h = nc.dram_tensor("wq", [C, 128], BF16, kind="ExternalInput")
    wk_h = nc.dram_tensor("wk", [C, 128], BF16, kind="ExternalInput")
    wv_h = nc.dram_tensor("wv", [C, 128], BF16, kind="ExternalInput")
    wp_h = nc.dram_tensor("wp", [C, C], BF16, kind="ExternalInput")
    bp_h = nc.dram_tensor("bp", [1, C], F32, kind="ExternalInput")
    out_h = nc.dram_tensor("out", [HPC, 512, 512], F32, kind="ExternalOutput")

    singles = tc.alloc_tile_pool(name="singles", bufs=1)
    epool = tc.alloc_tile_pool(name="epool", bufs=8)
    ojpool = tc.alloc_tile_pool(name="ojpool", bufs=2)
    vpool = tc.alloc_tile_pool(name="vpool", bufs=4)
    spool = tc.alloc_tile_pool(name="spool", bufs=2, space="PSUM")   # 6 banks
    opool = tc.alloc_tile_pool(name="opool", bufs=2, space="PSUM")   # 2 banks

    # ---- persistent SBUF tensors ----
    x_sb = singles.tile([128, CC, N], BF16)        # x[cc*128+p, n]
    wq_sb = singles.tile([128, CC, 128], BF16)
    wk_sb = singles.tile([128, CC, 128], BF16)
    wv_sb = singles.tile([128, CC, 128], BF16)
    wp_sb = singles.tile([128, CC, 512], BF16)
    bias_sb = singles.tile([128, 512], F32)
    id_sb = singles.tile([128, 128], BF16)
    kT_sb = singles.tile([128, N], BF16)           # [2*64 ch, n]
    vT_sb = singles.tile([128, N], BF16)
    # q token-major, both heads side by side, each with its own ones col:
    # cols [65h : 65h+64] = q of head h, col 65h+64 = 1.0
    qa_sb = singles.tile([128, NB, 130], BF16)
    # proj lhsT: mt[h][p, kk, jj]: contraction c = kk*128+p (n-offset inside
    # the jj%8-th 512-chunk of the channel), free jj = 8*ch + (n-chunk)
    mt_sb = [singles.tile([128, CC, 512], BF16, name=f"mt{h}") for h in range(HPC)]

    make_identity(nc, id_sb)
    nc.vector.memset(qa_sb[:, :, 64:65], 1.0)
    nc.vector.memset(qa_sb[:, :, 129:130], 1.0)

    # ---- input DMAs ----
    x_view = x_h.ap().rearrange("(cc p) n -> p cc n", p=128)

    def x_quarter(q8):
        # single descriptor per quarter: the sync engine issues DMAs
        # serially, so fewer/larger transfers keep the front DMA-fed
        nc.sync.dma_start(
            out=x_sb[:, :, ts(q8, N // 8)], in_=x_view[:, :, ts(q8, N // 8)]
        )

    nc.sync.dma_start(out=wk_sb, in_=wk_h.ap().rearrange("(cc p) m -> p cc m", p=128))
    x_quarter(0)
    nc.sync.dma_start(out=wv_sb, in_=wv_h.ap().rearrange("(cc p) m -> p cc m", p=128))
    x_quarter(1)
    nc.sync.dma_start(out=wq_sb, in_=wq_h.ap().rearrange("(cc p) m -> p cc m", p=128))
    for q8 in range(2, 8):
        x_quarter(q8)
    nc.sync.dma_start(out=wp_sb, in_=wp_h.ap().rearrange("(cc p) m -> p cc m", p=128))
    nc.sync.dma_start(out=bias_sb, in_=bp_h.ap().to_broadcast((128, 512)))

    # ---- PE warm-up ----
    # The HAM clock gate starts at half clock and needs ~3.4us of sustained
    # array activity to release.  Spin transposes of the identity (no data
    # deps beyond make_identity) while the x DMAs stream, so the KV phase
    # starts at full clock.
    for _ in range(30):
        wu_ps = spool.tile([128, 128], BF16, tag="s", name="wu_ps")
        nc.tensor.transpose(wu_ps, id_sb, id_sb)

    # ---- QKV phase ----
    # Packed per x-quarter so the PE always has work while the next
    # quarter's DMA is still in flight: K^T chunk, V^T chunk, then the
    # four Q m-blocks of the same quarter.
    for j8 in range(8):
        for w_sb, dst in ((wk_sb, kT_sb), (wv_sb, vT_sb)):
            kv_ps = spool.tile([128, 512], F32, tag="s", name="kv_ps")
            for cc in range(CC):
                nc.tensor.matmul(
                    kv_ps,
                    lhsT=w_sb[:, cc, :],
                    rhs=x_sb[:, cc, ts(j8, 512)],
                    start=(cc == 0),
                    stop=(cc == CC - 1),
                )
            nc.scalar.copy(out=dst[:, ts(j8, 512)], in_=kv_ps)
        for nb in range(4 * j8, 4 * j8 + 4):
            q_ps = spool.tile([128, 128], F32, tag="s", name="q_ps")
            for cc in range(CC):
                nc.tensor.matmul(
                    q_ps,
                    lhsT=x_sb[:, cc, ts(nb, 128)],
                    rhs=wq_sb[:, cc, :],
                    start=(cc == 0),
                    stop=(cc == CC - 1),
                )
            # [128, 2, 64] strided copy: head h -> qa cols [65h, 65h+64)
            nc.vector.tensor_copy(
                out=qa_sb[:, nb, :].rearrange("p (g c) -> p g c", g=2)[:, :, 0:64],
                in_=q_ps.rearrange("p (g c) -> p g c", g=2),
            )

    # ---- attention + norm ----
    # Baseline-proven stream structure: groups of GRP=3 (head, m-block)
    # tiles with heads alternating per tile (each S LDWEIGHTS targets the
    # other PE row group and hides under the current stream), one exp per
    # group, O matmuls lagged two groups.  On top of that: the O
    # accumulators are drained immediately at each chunk boundary and the
    # normalize tail (transposes + 1/Z) runs right there, so the next
    # chunk's stream starts without a long PE stall and the HAM clock
    # never re-throttles mid-attention.
    GRP = 3
    NT = 2 * NB
    n_grp = (NT + GRP - 1) // GRP
    eng_t = [0.0, 0.0]   # virtual ns on (ACT, DVE)

    def exp_to(e, s_ps, force=None):
        if force == 0 or (force is None and eng_t[0] <= eng_t[1]):
            eng_t[0] += ACT_EXP
            nc.scalar.activation(out=e, in_=s_ps, func=EXP)
        else:
            eng_t[1] += DVE_EXP
            nc.vector.tensor_scalar(
                out=e.bitcast(I16),
                in0=s_ps,
                scalar1=SCH_A,
                scalar2=SCH_B,
                op0=mybir.AluOpType.mult,
                op1=mybir.AluOpType.add,
            )

    def emit_tail(h, j, oj):
        # transposes of the drained O chunk + 1/Z application into mt
        t_ps = opool.tile([128, 4, 66], BF16, tag="o", name="t_ps")
        for q32 in range(4):
            nc.tensor.transpose(
                t_ps[:, q32, 0:65], oj[:, ts(q32, 128)], id_sb[0:65, 0:65]
            )
        rz4 = vpool.tile([128, 4], F32, tag="rz", name="rz4")
        nc.vector.reciprocal(out=rz4, in_=t_ps[:, :, 64:65])
        eng_t[1] += DVE_TAIL
        for q32 in range(4):
            dst = mt_sb[h][:, q32, :].rearrange("p (ch p8) -> p ch p8", p8=8)[:, :, j]
            nc.vector.tensor_scalar_mul(
                dst, t_ps[:, q32, 0:64], rz4[:, q32:q32 + 1]
            )

    NJ = 8
    pend_tails = []
    carry = None   # (o_ps, e_tiles, emit_o) of the previous chunk's last 4 groups
    for j in range(NJ):
        e_tiles = []
        o_ps = None

        def emit_o(g, _o, _e):
            g0, glen, pe = _e[g]
            for t in range(glen):
                k = g0 + t
                h, i = k % 2, k // 2
                nc.tensor.matmul(
                    _o[h][0:65, :],
                    lhsT=qa_sb[:, i, 65 * h:65 * h + 65],
                    rhs=pe[:, t, :],
                    start=(i == 0),
                    stop=(i == NB - 1),
                )

        for g in range(n_grp):
            g0 = g * GRP
            glen = min(GRP, NT - g0)
            s_ps = spool.tile([128, GRP, 512], F32, tag="s", name="s_ps")
            for t in range(glen):
                k = g0 + t
                h, i = k % 2, k // 2
                hb = h * HD
                nc.tensor.matmul(
                    s_ps[:, t, :],
                    lhsT=vT_sb[hb:hb + HD, ts(i, 128)],
                    rhs=kT_sb[hb:hb + HD, ts(j, 512)],
                    start=True,
                    stop=True,
                )
            e = epool.tile([128, GRP, 512], BF16, tag="e", name="e")
            if g == n_grp - 2:
                force = 1
            elif g == n_grp - 1:
                force = 0
            else:
                force = None
            exp_to(e[:, 0:glen, :], s_ps[:, 0:glen, :], force=force)
            e_tiles.append((g0, glen, e))
            # software pipeline across the chunk seam: the previous chunk's
            # last 4 O-groups and its PSUM drains slot in between this
            # chunk's first S bursts, so the in-order PE queue never idles
            # at a boundary.
            if g == 0 and carry is not None:
                po, pe_, pemit = carry
                pemit(n_grp - 4, po, pe_)
                pemit(n_grp - 3, po, pe_)
            elif g == 1 and carry is not None:
                po, pe_, pemit = carry
                pemit(n_grp - 2, po, pe_)
                pemit(n_grp - 1, po, pe_)
                for h in range(HPC):
                    oj = ojpool.tile([65, 512], BF16, tag="oj", name=f"oj{h}")
                    if h == 0:
                        eng_t[0] += ACT_OCP
                        nc.scalar.copy(out=oj, in_=po[h][0:65])
                    else:
                        eng_t[1] += DVE_OCP
                        nc.vector.tensor_copy(out=oj, in_=po[h][0:65])
                    pend_tails.append((h, j - 1, oj))
                carry = None
            elif g == 2 and pend_tails:
                for t_ in pend_tails:
                    emit_tail(*t_)
                pend_tails = []
            if g == 3:
                # after the tails' t_ps allocations, matching execution order
                o_ps = [opool.tile([128, 512], F32, tag="o", name=f"o_{h}")
                        for h in range(HPC)]
            if g >= 4 and g % 2 == 0:
                emit_o(g - 4, o_ps, e_tiles)
                emit_o(g - 3, o_ps, e_tiles)
        carry = (o_ps, e_tiles, emit_o)

    # flush the last chunk's carried work
    po, pe_, pemit = carry
    for gg in range(n_grp - 4, n_grp):
        pemit(gg, po, pe_)
    for h in range(HPC):
        oj = ojpool.tile([65, 512], BF16, tag="oj", name=f"oj{h}")
        eng_t[1 - h] += DVE_OCP if h == 0 else ACT_OCP
        if h == 0:
            nc.vector.tensor_copy(out=oj, in_=po[h][0:65])
        else:
            nc.scalar.copy(out=oj, in_=po[h][0:65])
        pend_tails.append((h, NJ - 1, oj))

    # ---- projection (interleaved with the last normalize tail) ----
    # y[jj, c''] = sum_c M[jj, c] w_proj[c'', c] + bp[c'']
    def emit_proj(h):
        for l in range(4):
            y_ps = spool.tile([128, 512], F32, tag="s", name="y_ps")
            for kk in range(CC):
                nc.tensor.matmul(
                    y_ps,
                    lhsT=mt_sb[h][:, kk, ts(l, 128)],
                    rhs=wp_sb[:, kk, :],
                    start=(kk == 0),
                    stop=(kk == CC - 1),
                )
            y_sb = vpool.tile([128, 512], F32, tag="y", name="y_sb")
            nc.vector.tensor_add(out=y_sb, in0=y_ps, in1=bias_sb)
            nc.sync.dma_start(out=out_h.ap()[h, ts(l, 128), :], in_=y_sb)

    emit_tail(*pend_tails.pop(0))
    emit_tail(*pend_tails.pop(0))
    emit_proj(0)
    emit_proj(1)

    for pool in (opool, spool, vpool, ojpool, epool, singles):
        pool.release()


_CACHE = {}


def _build():
    if "nc" not in _CACHE:
        nc = bacc.Bacc("TRN2", target_bir_lowering=False, debug=False)
        with tile.TileContext(nc) as tc:
            _emit(nc, tc)
        nc.compile()
        _CACHE["nc"] = nc
    return _CACHE["nc"]


def _shard(x, w_qkv, w_proj, b_proj):
    """Build the 8 per-core input maps from the full inputs."""
    bf16 = ml_dtypes.bfloat16
    wpT = np.ascontiguousarray(w_proj.T).astype(bf16)
    bp = np.ascontiguousarray(b_proj.reshape(1, C))
    in_maps = []
    for core in range(N_CORES):
        b = core // 4
        h0 = HPC * (core % 4)
        r0 = h0 * HD
        in_maps.append({
            "x": np.ascontiguousarray(x[b].reshape(C, N)).astype(bf16),
            "wq": np.ascontiguousarray(w_qkv[r0:r0 + 128, :].T).astype(bf16),
            "wk": np.ascontiguousarray(
                (w_qkv[C + r0:C + r0 + 128, :] * SCALE).T).astype(bf16),
            "wv": np.ascontiguousarray(
                w_qkv[2 * C + r0:2 * C + r0 + 128, :].T).astype(bf16),
            "wp": wpT,
            "bp": bp,
        })
    return in_maps


def _gather(results):
    full = np.empty((B, C, N), dtype=np.float32)
    for core in range(N_CORES):
        b = core // 4
        h0 = HPC * (core % 4)
        y = results[core]["out"]  # [2, 512, 512]
        for hi in range(HPC):
            ch0 = (h0 + hi) * HD
            full[b, ch0:ch0 + HD] = y[hi].reshape(HD, N)
    return full.reshape(B, C, H, W)


def run(inputs, trace=False, **kw):
    nc = _build()
    in_maps = _shard(**inputs)
    res = bass_utils.run_bass_kernel_spmd(
        nc, in_maps, core_ids=list(range(N_CORES)), trace=trace, **kw
    )
    return _gather(res.results), res


def kernel(x, w_qkv, w_proj, b_proj):
    out, _ = run(dict(x=x, w_qkv=w_qkv, w_proj=w_proj, b_proj=b_proj))
    return out
